# revision 65
# baseline (speedup 1.0000x reference)
"""Multi-head attention (B=2, N=2048, dim=1024, heads=16, dim_head=64) on
8 TRN2 NeuronCores.

Sharding: data-parallel over batch (2) x tensor-parallel over heads (4 per
core).  Core c handles batch b = c//4 and heads [4g, 4g+4), g = c%4.  Each
core computes its 4 heads' attention plus the partial out-projection
(O_heads @ w_out[head rows]); the host sums the 4 partials per batch and
adds the bias.

Per-core device algorithm (bf16 matmul inputs, fp32 PSUM accumulate; exp
without max-subtraction — scores are ~N(0,1) so exp never overflows):
  xT  [1024, 2048] = x[b].T               (transposed on host, free)
  Qt/Kt [128(i of head pair), 2048(n)] = w_slice.T @ xT    (W stationary)
  V   [128(m), 16(mt), 4(h), 65]  natural layout, col 64 = ones so the
       attention-value matmul also produces the softmax denominator.
  per head pair (A,B packed in PE rows 0-63 / 64-127 via tile_position),
  per 512-wide query chunk, per key tile mt:
       St[h] = Kt_h[64, 128].T @ Qt_h[64, 512] -> PSUM [128, 1024] (h0|h1
               in two adjacent banks of one wide tile)
       Pt = exp(St * 1/8)                      -> one [128, 1024] ScalarE
               activation covering both heads (halves ACT instruction count)
       Ot' += V'[128, 65].T @ Pt[:, 512h:+512] -> PSUM [65, 512] accum
  normalize: o = Ot'[0:64] * (1 / Ot'[64]) (DVE recip + GPSIMD bcast + DVE)
  proj: out[nt*128:+128, jc*512:+512] = sum_p o_sb[:,p,nt].T @ wo[:,p,jc]

Schedule highlights:
  - Host packs each input block (K weights, Q+V weights, chunk-major xT)
    into one contiguous [128, W] dram region -> six big 2D DMA ops across
    the two HWDGE rings (each ring allows ~4 outstanding ops, so op size
    sets in-flight bytes); the K block and xT chunk 0 land first so
    attention pair 0 starts ~24us in, with remaining Kt/Qt/V units as
    arrival-matched fillers inside the attention chunks.
  - ScalarE issues only the early xT-chunk-1 DMA: queued dma_starts would
    block the exp stream behind their ring-slot waits.
  - The AV lag queue carries across chunk boundaries (software
    pipelining): a chunk's last AVs and its normalize run in the next
    chunk's first slots, after its last exp has certainly retired.
  - Dummy matmuls cover every would-be PE idle window (startup DMA wait,
    pre-drain normalize latency): an idle PE trips the HAM clock gate to
    1.2 GHz for several us.
  - The drain's proj units rotate over the freed stp/ot/qk PSUM banks
    with ScalarE/DVE evacuation alternating, and the bf16 output rows
    split across both HWDGE rings.
"""
import numpy as np

import concourse.bass as bass
import concourse.mybir as mybir
import concourse.tile as tile
from concourse import bacc
from concourse.bass_utils import run_bass_kernel_spmd

# Problem constants (hardcoded per contract).
B = 2
N = 2048
DIM = 1024
HEADS = 16
DH = 64
INNER = HEADS * DH
SCALE = DH ** -0.5

N_CORES = 8
HEADS_PER_CORE = 4
PAIRS = 2          # head pairs per core
NT = N // 128      # 16 key/query tiles
DT = DIM // 128    # 8 contraction tiles
CH = N // 512      # 4 query chunks
F32 = mybir.dt.float32
F32R = mybir.dt.float32r
BF16 = mybir.dt.bfloat16

_CACHED_NC = None


def _emit_kernel(tc, xtp_d, wk_d, wqv_d, wo_d, out_d):
    nc = tc.nc

    from contextlib import ExitStack

    ctx = ExitStack()
    per = ctx.enter_context(tc.tile_pool(name="persist", bufs=1))
    psum = ctx.enter_context(tc.tile_pool(name="psum", bufs=1, space="PSUM"))
    work = ctx.enter_context(tc.tile_pool(name="work", bufs=1))

    # Persistent SBUF tensors.  xT is chunk-major [128, CH, DT, 512] and
    # the host packs each input tensor so every logical block (K weights,
    # one xT chunk, Q+V weights) is a single contiguous [128, W] dram
    # region — one big 2D DMA op each.  Arrival then follows issue order
    # with 4-8 KB contiguous rows (peak DMA efficiency), and the K block +
    # xT chunk 0 land first so attention can start ~16us in.
    xt_sb = per.tile([128, CH, DT, 512], BF16, tag="xt")
    wk_sb = per.tile([128, DT, 256], BF16, tag="wk")
    wqv_sb = per.tile([128, DT, 512], BF16, tag="wqv")
    wo_sb = per.tile([128, PAIRS, DIM], BF16, tag="wo")
    qt_sb = per.tile([128, PAIRS, N], BF16, tag="qt")
    kt_sb = per.tile([128, PAIRS, N], BF16, tag="kt")
    v_sb = per.tile([128, NT, HEADS_PER_CORE, DH + 1], BF16, tag="v")
    o_sb = per.tile([128, PAIRS, N], BF16, tag="o")

    # ScalarE (the second HWDGE ring) only issues the xT chunk-1 op: a
    # queued dma_start would block the exp stream behind its ring-slot
    # wait, so everything else goes on Sync in consumption order.
    nc.sync.dma_start(wk_sb[:], wk_d[:, :])
    nc.sync.dma_start(xt_sb[:, 0], xtp_d[:, 0:4096])
    nc.scalar.dma_start(xt_sb[:, 1], xtp_d[:, 4096:8192])
    nc.sync.dma_start(wqv_sb[:], wqv_d[:, :])
    nc.sync.dma_start(xt_sb[:, 2:4], xtp_d[:, 8192:16384])
    for p in range(PAIRS):
        nc.sync.dma_start(wo_sb[:, p, :], wo_d[128 * p:128 * (p + 1), :])

    # Ones column of V' (gives the softmax denominator through the AV matmul).
    ones_sb = per.tile([128, NT * HEADS_PER_CORE], F32, tag="ones")
    nc.vector.memset(ones_sb[:], 1.0)
    nc.vector.tensor_copy(
        v_sb[:, :, :, DH:DH + 1],
        ones_sb[:].rearrange("p (a b c) -> p a b c", b=HEADS_PER_CORE, c=1),
    )
    # Touch Exp once so the ACT table DMA (~1.3us + pseudo-load) happens
    # during the startup phase rather than before the first real exp.
    warm = work.tile([1, 1], F32, tag="warm")
    nc.scalar.activation(
        warm[:], ones_sb[0:1, 0:1], mybir.ActivationFunctionType.Exp, scale=1.0
    )

    def emit_dummies(n):
        # Keep the PE busy through DMA-only windows so the HAM clock gate
        # stays at full speed (an idle PE gets throttled to 1.2 GHz for
        # several us).  Output is never read.
        for _ in range(n):
            dmy = psum.tile([64, 64], F32, tag="stp", bufs=2, name="dummy")
            nc.tensor.matmul(
                dmy[:], ones_sb[:, 0:64], ones_sb[:, 0:64], start=True, stop=True
            )

    def emit_qk_chunk(which, p, c):
        """Qt or Kt for head pair p, n-chunk c: [128, 512] of W.T @ xT."""
        src = qt_sb if which == "q" else kt_sb
        w_sb = kt_w = wk_sb if which == "k" else wqv_sb
        col0 = 128 * p
        ps = psum.tile([128, 512], F32, tag="qk", bufs=2)
        for dt in range(DT):
            nc.tensor.matmul(
                ps[:],
                w_sb[:, dt, col0:col0 + 128],
                xt_sb[:, c, dt, :],
                start=(dt == 0),
                stop=(dt == DT - 1),
            )
        nc.vector.tensor_copy(src[:, p, 512 * c:512 * (c + 1)], ps[:])

    def emit_v_tile(mt, pr=None):
        """V natural [128(m), 4 heads x 64] for key tile mt.

        pr selects one head pair (half the stream work) so the pair-1
        half can move out of the PE-bound first attention chunk into the
        later ScalarE-bound chunks.
        """
        heads = HEADS_PER_CORE if pr is None else 2
        col0 = 256 + (0 if pr in (None, 0) else 128)
        h0 = 0 if pr in (None, 0) else 2
        mc, mo = divmod(mt, 4)
        ps = psum.tile([128, 64 * heads], F32, tag="qk", bufs=2, name="ps")
        for dt in range(DT):
            nc.tensor.matmul(
                ps[:],
                xt_sb[:, mc, dt, 128 * mo:128 * (mo + 1)],
                wqv_sb[:, dt, col0:col0 + 64 * heads],
                start=(dt == 0),
                stop=(dt == DT - 1),
            )
        nc.vector.tensor_copy(
            v_sb[:, mt, h0:h0 + heads, 0:DH],
            ps[:].rearrange("p (h d) -> p h d", h=heads),
        )

    ev_tiles = {}

    def emit_proj_unit(nt, jc, evac=None, psum_tag="qk"):
        """out[128nt:+128, 512jc:+512] = sum_p o_sb[:,p,nt].T @ wo[:,p,jc].

        Both jc halves of a row tile land in one [128, 1024] bf16 staging
        tile; the output DMA is split in two ops on two issuing engines so
        the tail transfer overlaps itself across DMA rings.
        """
        if nt not in ev_tiles:
            ev_tiles[nt] = work.tile([128, DIM], BF16, tag="ev", bufs=4, name="ev")
        ev = ev_tiles[nt]
        ps = psum.tile([128, 512], F32, tag=psum_tag, bufs=2, name="ps")
        for p in range(PAIRS):
            nc.tensor.matmul(
                ps[:],
                o_sb[:, p, 128 * nt:128 * (nt + 1)],
                wo_sb[:, p, 512 * jc:512 * (jc + 1)],
                start=(p == 0),
                stop=(p == PAIRS - 1),
            )
        if evac == "scalar":
            nc.scalar.copy(ev[:, 512 * jc:512 * (jc + 1)], ps[:])
        else:
            nc.vector.tensor_copy(ev[:, 512 * jc:512 * (jc + 1)], ps[:])
        if jc == 1:
            r0 = 128 * nt
            if evac is None:
                nc.sync.dma_start(out_d[r0:r0 + 128, :], ev[:])
            else:
                # Drain: split by rows across both HWDGE engines so the
                # final transfers overlap across DMA rings.
                nc.sync.dma_start(out_d[r0:r0 + 64, :], ev[0:64, :])
                nc.scalar.dma_start(out_d[r0 + 64:r0 + 128, :], ev[64:128, :])
            del ev_tiles[nt]

    def emit_unit(u):
        if u[0] == "v":
            emit_v_tile(u[1], u[2] if len(u) > 2 else None)
        elif u[0] == "qk":
            emit_qk_chunk(u[1], u[2], u[3])
        elif u[0] == "proj":
            emit_proj_unit(u[1], u[2])
        elif u[0] == "dummy":
            emit_dummies(u[1])

    def emit_normalize(ot, p, c, engine="vector"):
        """Normalize: o = Ot'[0:64] / Ot'[64].  First evacuate each head's
        Ot' to SBUF (denominator row to a partition-0 tile, numerator via
        one wide copy) — that frees the ot PSUM bank after ~1.1us so the
        next chunk's first AV matmul never waits on the rest of the chain
        (recip/broadcast/mul).  engine="scalar" moves the copies to the
        otherwise-idle ScalarE during the drain.

        (reciprocal_approx_fast misbehaves on hardware when its input AP
        sits at a nonzero base partition, so den gets a partition-0 copy.)
        """
        # engine="scalar" (drain only): den copies go to the idle ScalarE
        # while the otmp copies stay on DVE, so the two evacuations run in
        # parallel and the recip chain starts ~1.2us sooner.
        den_copy = (nc.scalar.copy if engine == "scalar"
                    else lambda o, i: nc.vector.tensor_copy(o, i))
        den, otmp, recip, rbc = ([None, None] for _ in range(4))
        for h in range(2):
            den[h] = work.tile([1, 512], F32, tag="den", bufs=4, name=f"den{h}")
            den_copy(den[h][:], ot[h][DH:DH + 1, :])
            otmp[h] = work.tile([DH, 512], F32, tag="otmp", bufs=4, name=f"otmp{h}")
            nc.vector.tensor_copy(otmp[h][:], ot[h][0:DH, :])
        for h in range(2):
            recip[h] = work.tile([1, 512], F32, tag="recip", bufs=4, name=f"rec{h}")
            nc.vector.reciprocal_approx_fast(recip[h][:], den[h][:])
        for h in range(2):
            if engine == "scalar":
                # Drain: the PE is idle and PSUM banks are freeing up, so
                # broadcast via a K=1 outer product (ones^T @ recip,
                # ~0.2us) instead of the 1us serial GpSimd broadcast —
                # this chain gates the final proj row tiles.
                rbc[h] = psum.tile([64, 512], F32, tag="ot", bufs=2, name="rbcps")
                nc.tensor.matmul(
                    rbc[h][:], ones_sb[0:1, 0:DH], recip[h][:],
                    start=True, stop=True,
                )
            else:
                rbc[h] = work.tile([64, 512], F32, tag="rbc", bufs=4, name=f"rbc{h}")
                nc.gpsimd.partition_broadcast(rbc[h][:], recip[h][:])
        for h in range(2):
            nc.vector.tensor_mul(
                o_sb[64 * h:64 * (h + 1), p, 512 * c:512 * (c + 1)],
                otmp[h][:],
                rbc[h][:],
            )

    # AV matmuls lag the St/exp of the current key tile by two slots so the
    # in-order PE queue never head-of-line blocks on ScalarE.  The pending
    # list carries across chunk boundaries (software pipelining): a chunk's
    # last two AVs and its normalize are emitted during the next chunk's
    # first two slots, by which time its last exp has certainly finished.
    pending = []

    def flush_one(last_norm_engine="vector"):
        e = pending.pop(0)
        for h in range(2):
            nc.tensor.matmul(
                e["ot"][h][:],
                v_sb[:, e["mt"], 2 * e["p"] + h, :],
                e["pt"][:, 512 * h:512 * (h + 1)],
                start=(e["mt"] == 0),
                stop=(e["mt"] == NT - 1),
            )
        if e["mt"] == NT - 1:
            emit_normalize(e["ot"], e["p"], e["c"], engine=last_norm_engine)

    def emit_att_chunk(p, c, filler=None):
        """Attention for head pair p, query chunk c (cols 512c..512c+512).

        The two heads of a pair sit in PE rows 0-63 / 64-127 (tile_position
        row packing); their St outputs land in the two adjacent PSUM banks
        of one [128, 1024] tile so a single wide ScalarE activation
        exponentiates both.
        """
        ot = [
            psum.tile([DH + 1, 512], F32, tag="ot", bufs=2, name=f"ot{h}")
            for h in range(2)
        ]
        for mt in range(NT):
            stp = psum.tile([128, 1024], F32, tag="stp", bufs=2, name="stp")
            for h in range(2):
                nc.tensor.matmul(
                    stp[:, 512 * h:512 * (h + 1)],
                    kt_sb[64 * h:64 * (h + 1), p, 128 * mt:128 * (mt + 1)],
                    qt_sb[64 * h:64 * (h + 1), p, 512 * c:512 * (c + 1)],
                    start=True,
                    stop=True,
                    tile_position=(64 * h, 0),
                )
            pt = work.tile([128, 1024], BF16, tag="pt", bufs=8, name="pt")
            nc.scalar.activation(
                pt[:], stp[:], mybir.ActivationFunctionType.Exp, scale=SCALE
            )
            pending.append({"ot": ot, "p": p, "c": c, "mt": mt, "pt": pt})
            if len(pending) == 4:
                flush_one()
            if filler is not None:
                filler(c, mt)

    # ---- Emission schedule ----
    # Phase B: dummy matmuls cover the initial DMA window (wqkv + xT chunk
    # 0 land ~8us in), then Kt pair 0 per xT-chunk arrival (the critical
    # path to attention start), Qt(p0, c0), and the first two V tiles.
    # Dummies bridge the xT chunk arrival gaps (the PE would otherwise
    # idle on the DMA semaphore and trip the HAM clock throttle).
    # Attention starts right after Kt(p0) chunks 0-1; chunks 2-3 stream in
    # as chunk-0 fillers, arrival-matched to their St consumers.
    phase_b = [
        ("dummy", 26),
        ("qk", "k", 0, 0), ("dummy", 10),
        ("qk", "q", 0, 0), ("v", 0), ("v", 1),
    ]
    for u in phase_b:
        emit_unit(u)

    # Fillers (slot = mt index, NT = after last exp).  Fillers sit at the
    # START of each chunk: the first AV of a chunk waits ~1.2us for the
    # previous chunk's ot evacuation, and St(mt=1) waits for the previous
    # chunk's last exp to free its stp slot — front-loaded fillers absorb
    # both so the PE never idles (an idle PE trips the HAM clock throttle).
    # NOTE: inside emit_att_chunk the AV flush for key tiles (mt-2, mt-1)
    # is emitted BEFORE the slot-mt filler, so V tile j must sit at slot
    # <= j+1 (one earlier to cover the DVE evac latency).
    att0_fill = {
        0: {0: [("qk", "k", 0, 1), ("v", 2)],
            1: [("v", 3), ("v", 4)], 2: [("v", 5)],
            3: [("qk", "k", 0, 2)], 4: [("v", 6), ("v", 7)],
            5: [("v", 8)], 6: [("v", 9)], 7: [("qk", "k", 0, 3)],
            8: [("v", 10), ("v", 11)], 10: [("v", 12)],
            11: [("v", 13)], 12: [("v", 14)],
            13: [("v", 15), ("qk", "q", 0, 1)]},
        1: {0: [("qk", "k", 1, 0)], 1: [("qk", "q", 0, 2)],
            8: [("qk", "k", 1, 1)]},
        2: {0: [("qk", "k", 1, 2)], 1: [("qk", "q", 0, 3)],
            8: [("qk", "q", 1, 0)]},
        3: {0: [("qk", "k", 1, 3)], 8: [("qk", "q", 1, 1)]},
    }

    def att0_filler(c, mt):
        for u in att0_fill[c].get(mt, ()):
            emit_unit(u)

    for c in range(CH):
        emit_att_chunk(0, c, filler=att0_filler)

    # Phase D: attention pair 1.  Chunk 0 finishes the last Qt unit;
    # chunks 1-3 carry the out-projection for the query rows of chunk c-1
    # (complete for both pairs by then), front-loaded for the same reason.
    # proj fillers start at slot 3: the previous chunk's normalize is only
    # emitted during this chunk's slot-2 flush, and proj reads its o rows.
    # Slots 0-2 already carry the previous chunk's final AVs + normalize,
    # so proj spreads over the middle to keep the exp stream dense.
    att1_fill = {
        0: {0: [("qk", "q", 1, 2)], 8: [("qk", "q", 1, 3)]},
    }
    for c in range(1, CH):
        units = [("proj", nt, jc)
                 for nt in range(4 * (c - 1), 4 * c) for jc in range(2)]
        slots = [3, 5, 7, 9, 10, 11, 12, 13]
        att1_fill[c] = {}
        for s, u in zip(slots, units):
            att1_fill[c].setdefault(s, []).append(u)

    def att1_filler(c, mt):
        for u in att1_fill.get(c, {}).get(mt, ()):
            emit_unit(u)

    for c in range(CH):
        emit_att_chunk(1, c, filler=att1_filler)
    # Drain the carried AVs of the last chunk; its normalize copies go to
    # ScalarE (idle after the last exp) so the DVE backlog of proj
    # evacuations never delays the final proj units.
    while pending:
        flush_one(last_norm_engine="scalar")

    # Drain: last four row tiles.  The attention PSUM banks (stp/ot) are
    # free now, so the proj units rotate over three tags (6 slots) and the
    # evacuations alternate between ScalarE and DVE — both pipeline instead
    # of serializing on two qk slots.  A few dummies bridge the last
    # normalize's latency so the PE never idles into a HAM throttle.
    emit_dummies(14)
    tags = ["qk", "stp", "ot"]
    i = 0
    for nt in range(12, 16):
        for jc in range(2):
            emit_proj_unit(
                nt, jc, evac="scalar" if i % 2 else None, psum_tag=tags[i % 3]
            )
            i += 1

    ctx.close()


def _build():
    global _CACHED_NC
    if _CACHED_NC is not None:
        return _CACHED_NC
    nc = bacc.Bacc(
        "TRN2",
        target_bir_lowering=False,
        debug=False,
        enable_asserts=True,
        num_devices=N_CORES,
    )
    xtp_d = nc.dram_tensor("xtp", [128, CH * DT * 512], BF16,
                           kind="ExternalInput").ap()
    wk_d = nc.dram_tensor("wk", [128, DT * 256], BF16, kind="ExternalInput").ap()
    wqv_d = nc.dram_tensor("wqv", [128, DT * 512], BF16,
                           kind="ExternalInput").ap()
    wo_d = nc.dram_tensor("wo", [256, DIM], BF16, kind="ExternalInput").ap()
    out_d = nc.dram_tensor("out", [N, DIM], BF16, kind="ExternalOutput").ap()

    with tile.TileContext(nc) as tc:
        _emit_kernel(tc, xtp_d, wk_d, wqv_d, wo_d, out_d)
    nc.compile()
    _CACHED_NC = nc
    return nc


def _in_maps(x, w_qkv, w_out):
    import ml_dtypes

    bf = ml_dtypes.bfloat16

    def pack(block):
        # [1024, W] (contraction-major) -> [128, DT*W]: row p holds the
        # per-dt blocks back to back, matching the SBUF [128, DT, W] tiles.
        w = block.shape[1]
        return block.reshape(DT, 128, w).transpose(1, 0, 2).reshape(128, DT * w)

    maps = []
    for c in range(N_CORES):
        b, g = divmod(c, 4)
        cols = slice(256 * g, 256 * (g + 1))
        xt = x[b].T  # [DIM, N]
        xtp = np.concatenate(
            [pack(xt[:, 512 * ch:512 * (ch + 1)]) for ch in range(CH)], axis=1
        )
        wqv = np.concatenate(
            [w_qkv[:, cols], w_qkv[:, 2 * INNER:][:, cols]], axis=1
        )
        maps.append(
            {
                "xtp": np.ascontiguousarray(xtp.astype(bf)),
                "wk": np.ascontiguousarray(
                    pack(w_qkv[:, INNER:][:, cols]).astype(bf)
                ),
                "wqv": np.ascontiguousarray(pack(wqv).astype(bf)),
                "wo": np.ascontiguousarray(w_out[cols, :].astype(bf)),
            }
        )
    return maps


def _run(x, w_qkv, w_out, b_out, trace=False):
    nc = _build()
    res = run_bass_kernel_spmd(
        nc, _in_maps(x, w_qkv, w_out), list(range(N_CORES)), trace=trace
    )
    partials = np.stack(
        [np.asarray(res.results[c]["out"], dtype=np.float32)
         for c in range(N_CORES)]
    )
    out = np.empty((B, N, DIM), dtype=np.float32)
    for b in range(B):
        out[b] = partials[4 * b:4 * b + 4].sum(axis=0) + b_out
    return out, res


def kernel(x, w_qkv, w_out, b_out):
    out, _ = _run(
        np.asarray(x, dtype=np.float32),
        np.asarray(w_qkv, dtype=np.float32),
        np.asarray(w_out, dtype=np.float32),
        np.asarray(b_out, dtype=np.float32),
    )
    return out


# revision 68
# speedup vs baseline: 1.0010x; 1.0010x over previous
"""Multi-head attention (B=2, N=2048, dim=1024, heads=16, dim_head=64) on
8 TRN2 NeuronCores.

Sharding: data-parallel over batch (2) x tensor-parallel over heads (4 per
core).  Core c handles batch b = c//4 and heads [4g, 4g+4), g = c%4.  Each
core computes its 4 heads' attention plus the partial out-projection
(O_heads @ w_out[head rows]); the host sums the 4 partials per batch and
adds the bias.

Per-core device algorithm (bf16 matmul inputs, fp32 PSUM accumulate; exp
without max-subtraction — scores are ~N(0,1) so exp never overflows):
  xT  [1024, 2048] = x[b].T               (transposed on host, free)
  Qt/Kt [128(i of head pair), 2048(n)] = w_slice.T @ xT    (W stationary)
  V   [128(m), 16(mt), 4(h), 65]  natural layout, col 64 = ones so the
       attention-value matmul also produces the softmax denominator.
  per head pair (A,B packed in PE rows 0-63 / 64-127 via tile_position),
  per 512-wide query chunk, per key tile mt:
       St[h] = Kt_h[64, 128].T @ Qt_h[64, 512] -> PSUM [128, 1024] (h0|h1
               in two adjacent banks of one wide tile)
       Pt = exp(St * 1/8)                      -> one [128, 1024] ScalarE
               activation covering both heads (halves ACT instruction count)
       Ot' += V'[128, 65].T @ Pt[:, 512h:+512] -> PSUM [65, 512] accum
  normalize: o = Ot'[0:64] * (1 / Ot'[64]) (DVE recip + GPSIMD bcast + DVE)
  proj: out[nt*128:+128, jc*512:+512] = sum_p o_sb[:,p,nt].T @ wo[:,p,jc]

Schedule highlights:
  - Host packs each input block (K weights, Q+V weights, chunk-major xT)
    into one contiguous [128, W] dram region -> six big 2D DMA ops across
    the two HWDGE rings (each ring allows ~4 outstanding ops, so op size
    sets in-flight bytes); the K block and xT chunk 0 land first so
    attention pair 0 starts ~24us in, with remaining Kt/Qt/V units as
    arrival-matched fillers inside the attention chunks.
  - ScalarE issues only the early xT-chunk-1 DMA: queued dma_starts would
    block the exp stream behind their ring-slot waits.
  - The AV lag queue carries across chunk boundaries (software
    pipelining): a chunk's last AVs and its normalize run in the next
    chunk's first slots, after its last exp has certainly retired.
  - Dummy matmuls cover every would-be PE idle window (startup DMA wait,
    pre-drain normalize latency): an idle PE trips the HAM clock gate to
    1.2 GHz for several us.
  - The drain's proj units rotate over the freed stp/ot/qk PSUM banks
    with ScalarE/DVE evacuation alternating, and the bf16 output rows
    split across both HWDGE rings.
"""
import numpy as np

import concourse.bass as bass
import concourse.mybir as mybir
import concourse.tile as tile
from concourse import bacc
from concourse.bass_utils import run_bass_kernel_spmd

# Problem constants (hardcoded per contract).
B = 2
N = 2048
DIM = 1024
HEADS = 16
DH = 64
INNER = HEADS * DH
SCALE = DH ** -0.5

N_CORES = 8
HEADS_PER_CORE = 4
PAIRS = 2          # head pairs per core
NT = N // 128      # 16 key/query tiles
DT = DIM // 128    # 8 contraction tiles
CH = N // 512      # 4 query chunks
F32 = mybir.dt.float32
F32R = mybir.dt.float32r
BF16 = mybir.dt.bfloat16

_CACHED_NC = None


def _emit_kernel(tc, xtp_d, wk_d, wqv_d, wo_d, out_d):
    nc = tc.nc

    from contextlib import ExitStack

    ctx = ExitStack()
    per = ctx.enter_context(tc.tile_pool(name="persist", bufs=1))
    psum = ctx.enter_context(tc.tile_pool(name="psum", bufs=1, space="PSUM"))
    work = ctx.enter_context(tc.tile_pool(name="work", bufs=1))

    # Persistent SBUF tensors.  xT is chunk-major [128, CH, DT, 512] and
    # the host packs each input tensor so every logical block (K weights,
    # one xT chunk, Q+V weights) is a single contiguous [128, W] dram
    # region — one big 2D DMA op each.  Arrival then follows issue order
    # with 4-8 KB contiguous rows (peak DMA efficiency), and the K block +
    # xT chunk 0 land first so attention can start ~16us in.
    xt_sb = per.tile([128, CH, DT, 512], BF16, tag="xt")
    wk_sb = per.tile([128, DT, 256], BF16, tag="wk")
    wqv_sb = per.tile([128, DT, 512], BF16, tag="wqv")
    wo_sb = per.tile([128, PAIRS, DIM], BF16, tag="wo")
    qt_sb = per.tile([128, PAIRS, N], BF16, tag="qt")
    kt_sb = per.tile([128, PAIRS, N], BF16, tag="kt")
    v_sb = per.tile([128, NT, HEADS_PER_CORE, DH + 1], BF16, tag="v")
    o_sb = per.tile([128, PAIRS, N], BF16, tag="o")

    # ScalarE (the second HWDGE ring) only issues the xT chunk-1 op: a
    # queued dma_start would block the exp stream behind its ring-slot
    # wait, so everything else goes on Sync in consumption order.
    nc.sync.dma_start(wk_sb[:], wk_d[:, :])
    nc.sync.dma_start(xt_sb[:, 0], xtp_d[:, 0:4096])
    nc.scalar.dma_start(xt_sb[:, 1], xtp_d[:, 4096:8192])
    nc.sync.dma_start(wqv_sb[:], wqv_d[:, :])
    nc.sync.dma_start(xt_sb[:, 2:4], xtp_d[:, 8192:16384])
    for p in range(PAIRS):
        nc.sync.dma_start(wo_sb[:, p, :], wo_d[128 * p:128 * (p + 1), :])

    # Ones column of V' (gives the softmax denominator through the AV matmul).
    ones_sb = per.tile([128, NT * HEADS_PER_CORE], F32, tag="ones")
    nc.vector.memset(ones_sb[:], 1.0)
    nc.vector.tensor_copy(
        v_sb[:, :, :, DH:DH + 1],
        ones_sb[:].rearrange("p (a b c) -> p a b c", b=HEADS_PER_CORE, c=1),
    )
    # Touch Exp once so the ACT table DMA (~1.3us + pseudo-load) happens
    # during the startup phase rather than before the first real exp.
    warm = work.tile([1, 1], F32, tag="warm")
    nc.scalar.activation(
        warm[:], ones_sb[0:1, 0:1], mybir.ActivationFunctionType.Exp, scale=1.0
    )

    def emit_dummies(n):
        # Keep the PE busy through DMA-only windows so the HAM clock gate
        # stays at full speed (an idle PE gets throttled to 1.2 GHz for
        # several us).  Output is never read.
        for _ in range(n):
            dmy = psum.tile([64, 64], F32, tag="stp", bufs=2, name="dummy")
            nc.tensor.matmul(
                dmy[:], ones_sb[:, 0:64], ones_sb[:, 0:64], start=True, stop=True
            )

    def emit_qk_chunk(which, p, c):
        """Qt or Kt for head pair p, n-chunk c: [128, 512] of W.T @ xT."""
        src = qt_sb if which == "q" else kt_sb
        w_sb = kt_w = wk_sb if which == "k" else wqv_sb
        col0 = 128 * p
        ps = psum.tile([128, 512], F32, tag="qk", bufs=2)
        for dt in range(DT):
            nc.tensor.matmul(
                ps[:],
                w_sb[:, dt, col0:col0 + 128],
                xt_sb[:, c, dt, :],
                start=(dt == 0),
                stop=(dt == DT - 1),
            )
        nc.vector.tensor_copy(src[:, p, 512 * c:512 * (c + 1)], ps[:])

    def emit_v_tile(mt, pr=None):
        """V natural [128(m), 4 heads x 64] for key tile mt.

        pr selects one head pair (half the stream work) so the pair-1
        half can move out of the PE-bound first attention chunk into the
        later ScalarE-bound chunks.
        """
        heads = HEADS_PER_CORE if pr is None else 2
        col0 = 256 + (0 if pr in (None, 0) else 128)
        h0 = 0 if pr in (None, 0) else 2
        mc, mo = divmod(mt, 4)
        ps = psum.tile([128, 64 * heads], F32, tag="qk", bufs=2, name="ps")
        for dt in range(DT):
            nc.tensor.matmul(
                ps[:],
                xt_sb[:, mc, dt, 128 * mo:128 * (mo + 1)],
                wqv_sb[:, dt, col0:col0 + 64 * heads],
                start=(dt == 0),
                stop=(dt == DT - 1),
            )
        nc.vector.tensor_copy(
            v_sb[:, mt, h0:h0 + heads, 0:DH],
            ps[:].rearrange("p (h d) -> p h d", h=heads),
        )

    ev_tiles = {}

    def emit_proj_unit(nt, jc, evac=None, psum_tag="qk"):
        """out[128nt:+128, 512jc:+512] = sum_p o_sb[:,p,nt].T @ wo[:,p,jc].

        Both jc halves of a row tile land in one [128, 1024] bf16 staging
        tile; the output DMA is split in two ops on two issuing engines so
        the tail transfer overlaps itself across DMA rings.
        """
        if nt not in ev_tiles:
            ev_tiles[nt] = work.tile([128, DIM], BF16, tag="ev", bufs=4, name="ev")
        ev = ev_tiles[nt]
        ps = psum.tile([128, 512], F32, tag=psum_tag, bufs=2, name="ps")
        for p in range(PAIRS):
            nc.tensor.matmul(
                ps[:],
                o_sb[:, p, 128 * nt:128 * (nt + 1)],
                wo_sb[:, p, 512 * jc:512 * (jc + 1)],
                start=(p == 0),
                stop=(p == PAIRS - 1),
            )
        if evac == "scalar":
            nc.scalar.copy(ev[:, 512 * jc:512 * (jc + 1)], ps[:])
        else:
            nc.vector.tensor_copy(ev[:, 512 * jc:512 * (jc + 1)], ps[:])
        if jc == 1:
            r0 = 128 * nt
            if evac is None:
                nc.sync.dma_start(out_d[r0:r0 + 128, :], ev[:])
            else:
                # Drain: split by rows across both HWDGE engines so the
                # final transfers overlap across DMA rings.
                nc.sync.dma_start(out_d[r0:r0 + 64, :], ev[0:64, :])
                nc.scalar.dma_start(out_d[r0 + 64:r0 + 128, :], ev[64:128, :])
            del ev_tiles[nt]

    def emit_unit(u):
        if u[0] == "v":
            emit_v_tile(u[1], u[2] if len(u) > 2 else None)
        elif u[0] == "qk":
            emit_qk_chunk(u[1], u[2], u[3])
        elif u[0] == "proj":
            emit_proj_unit(u[1], u[2])
        elif u[0] == "dummy":
            emit_dummies(u[1])

    def emit_normalize(ot, p, c, engine="vector"):
        """Normalize: o = Ot'[0:64] / Ot'[64].  First evacuate each head's
        Ot' to SBUF (denominator row to a partition-0 tile, numerator via
        one wide copy) — that frees the ot PSUM bank after ~1.1us so the
        next chunk's first AV matmul never waits on the rest of the chain
        (recip/broadcast/mul).  engine="scalar" moves the copies to the
        otherwise-idle ScalarE during the drain.

        (reciprocal_approx_fast misbehaves on hardware when its input AP
        sits at a nonzero base partition, so den gets a partition-0 copy.)
        """
        # engine="scalar" (drain only): den copies go to the idle ScalarE
        # while the otmp copies stay on DVE, so the two evacuations run in
        # parallel and the recip chain starts ~1.2us sooner.
        den_copy = (nc.scalar.copy if engine == "scalar"
                    else lambda o, i: nc.vector.tensor_copy(o, i))
        den, otmp, recip, rbc = ([None, None] for _ in range(4))
        for h in range(2):
            den[h] = work.tile([1, 512], F32, tag="den", bufs=4, name=f"den{h}")
            den_copy(den[h][:], ot[h][DH:DH + 1, :])
            otmp[h] = work.tile([DH, 512], F32, tag="otmp", bufs=4, name=f"otmp{h}")
            nc.vector.tensor_copy(otmp[h][:], ot[h][0:DH, :])
        for h in range(2):
            recip[h] = work.tile([1, 512], F32, tag="recip", bufs=4, name=f"rec{h}")
            nc.vector.reciprocal_approx_fast(recip[h][:], den[h][:])
        for h in range(2):
            rbc[h] = work.tile([64, 512], F32, tag="rbc", bufs=4, name=f"rbc{h}")
            nc.gpsimd.partition_broadcast(rbc[h][:], recip[h][:])
        for h in range(2):
            nc.vector.tensor_mul(
                o_sb[64 * h:64 * (h + 1), p, 512 * c:512 * (c + 1)],
                otmp[h][:],
                rbc[h][:],
            )

    # AV matmuls lag the St/exp of the current key tile by two slots so the
    # in-order PE queue never head-of-line blocks on ScalarE.  The pending
    # list carries across chunk boundaries (software pipelining): a chunk's
    # last two AVs and its normalize are emitted during the next chunk's
    # first two slots, by which time its last exp has certainly finished.
    pending = []

    def flush_one(last_norm_engine="vector"):
        e = pending.pop(0)
        for h in range(2):
            nc.tensor.matmul(
                e["ot"][h][:],
                v_sb[:, e["mt"], 2 * e["p"] + h, :],
                e["pt"][:, 512 * h:512 * (h + 1)],
                start=(e["mt"] == 0),
                stop=(e["mt"] == NT - 1),
            )
        if e["mt"] == NT - 1:
            emit_normalize(e["ot"], e["p"], e["c"], engine=last_norm_engine)

    def emit_att_chunk(p, c, filler=None):
        """Attention for head pair p, query chunk c (cols 512c..512c+512).

        The two heads of a pair sit in PE rows 0-63 / 64-127 (tile_position
        row packing); their St outputs land in the two adjacent PSUM banks
        of one [128, 1024] tile so a single wide ScalarE activation
        exponentiates both.
        """
        ot = [
            psum.tile([DH + 1, 512], F32, tag="ot", bufs=2, name=f"ot{h}")
            for h in range(2)
        ]
        for mt in range(NT):
            stp = psum.tile([128, 1024], F32, tag="stp", bufs=2, name="stp")
            for h in range(2):
                nc.tensor.matmul(
                    stp[:, 512 * h:512 * (h + 1)],
                    kt_sb[64 * h:64 * (h + 1), p, 128 * mt:128 * (mt + 1)],
                    qt_sb[64 * h:64 * (h + 1), p, 512 * c:512 * (c + 1)],
                    start=True,
                    stop=True,
                    tile_position=(64 * h, 0),
                )
            pt = work.tile([128, 1024], BF16, tag="pt", bufs=8, name="pt")
            nc.scalar.activation(
                pt[:], stp[:], mybir.ActivationFunctionType.Exp, scale=SCALE
            )
            pending.append({"ot": ot, "p": p, "c": c, "mt": mt, "pt": pt})
            if len(pending) == 4:
                flush_one()
            if filler is not None:
                filler(c, mt)

    # ---- Emission schedule ----
    # Phase B: dummy matmuls cover the initial DMA window (wqkv + xT chunk
    # 0 land ~8us in), then Kt pair 0 per xT-chunk arrival (the critical
    # path to attention start), Qt(p0, c0), and the first two V tiles.
    # Dummies bridge the xT chunk arrival gaps (the PE would otherwise
    # idle on the DMA semaphore and trip the HAM clock throttle).
    # Attention starts right after Kt(p0) chunks 0-1; chunks 2-3 stream in
    # as chunk-0 fillers, arrival-matched to their St consumers.
    phase_b = [
        ("dummy", 26),
        ("qk", "k", 0, 0), ("dummy", 10),
        ("qk", "q", 0, 0),
    ]
    for u in phase_b:
        emit_unit(u)

    # Fillers (slot = mt index, NT = after last exp).  Fillers sit at the
    # START of each chunk: the first AV of a chunk waits ~1.2us for the
    # previous chunk's ot evacuation, and St(mt=1) waits for the previous
    # chunk's last exp to free its stp slot — front-loaded fillers absorb
    # both so the PE never idles (an idle PE trips the HAM clock throttle).
    # NOTE: inside emit_att_chunk the AV flush for key tiles (mt-2, mt-1)
    # is emitted BEFORE the slot-mt filler, so V tile j must sit at slot
    # <= j+1 (one earlier to cover the DVE evac latency).
    att0_fill = {
        0: {0: [("v", 0), ("v", 1), ("qk", "k", 0, 1)],
            1: [("v", 2), ("v", 3), ("v", 4)], 2: [("v", 5)],
            3: [("qk", "k", 0, 2)], 4: [("v", 6), ("v", 7)],
            5: [("v", 8)], 6: [("v", 9)], 7: [("qk", "k", 0, 3)],
            8: [("v", 10), ("v", 11)], 10: [("v", 12)],
            11: [("v", 13)], 12: [("v", 14)],
            13: [("v", 15), ("qk", "q", 0, 1)]},
        1: {0: [("qk", "k", 1, 0)], 1: [("qk", "q", 0, 2)],
            8: [("qk", "k", 1, 1)]},
        2: {0: [("qk", "k", 1, 2)], 1: [("qk", "q", 0, 3)],
            8: [("qk", "q", 1, 0)]},
        3: {0: [("qk", "k", 1, 3)], 8: [("qk", "q", 1, 1)]},
    }

    def att0_filler(c, mt):
        for u in att0_fill[c].get(mt, ()):
            emit_unit(u)

    for c in range(CH):
        emit_att_chunk(0, c, filler=att0_filler)

    # Phase D: attention pair 1.  Chunk 0 finishes the last Qt unit;
    # chunks 1-3 carry the out-projection for the query rows of chunk c-1
    # (complete for both pairs by then), front-loaded for the same reason.
    # proj fillers start at slot 3: the previous chunk's normalize is only
    # emitted during this chunk's slot-2 flush, and proj reads its o rows.
    # Slots 0-2 already carry the previous chunk's final AVs + normalize,
    # so proj spreads over the middle to keep the exp stream dense.
    att1_fill = {
        0: {0: [("qk", "q", 1, 2)], 8: [("qk", "q", 1, 3)]},
    }
    for c in range(1, CH):
        units = [("proj", nt, jc)
                 for nt in range(4 * (c - 1), 4 * c) for jc in range(2)]
        slots = [3, 5, 7, 9, 10, 11, 12, 13]
        att1_fill[c] = {}
        for s, u in zip(slots, units):
            att1_fill[c].setdefault(s, []).append(u)

    def att1_filler(c, mt):
        for u in att1_fill.get(c, {}).get(mt, ()):
            emit_unit(u)

    for c in range(CH):
        emit_att_chunk(1, c, filler=att1_filler)
    # Drain the carried AVs of the last chunk; its normalize copies go to
    # ScalarE (idle after the last exp) so the DVE backlog of proj
    # evacuations never delays the final proj units.
    while pending:
        flush_one(last_norm_engine="scalar")

    # Drain: last four row tiles.  The attention PSUM banks (stp/ot) are
    # free now, so the proj units rotate over three tags (6 slots) and the
    # evacuations alternate between ScalarE and DVE — both pipeline instead
    # of serializing on two qk slots.  A few dummies bridge the last
    # normalize's latency so the PE never idles into a HAM throttle.
    emit_dummies(14)
    tags = ["qk", "stp", "ot"]
    i = 0
    for nt in range(12, 16):
        for jc in range(2):
            emit_proj_unit(
                nt, jc, evac="scalar" if i % 2 else None, psum_tag=tags[i % 3]
            )
            i += 1

    ctx.close()


def _build():
    global _CACHED_NC
    if _CACHED_NC is not None:
        return _CACHED_NC
    nc = bacc.Bacc(
        "TRN2",
        target_bir_lowering=False,
        debug=False,
        enable_asserts=True,
        num_devices=N_CORES,
    )
    xtp_d = nc.dram_tensor("xtp", [128, CH * DT * 512], BF16,
                           kind="ExternalInput").ap()
    wk_d = nc.dram_tensor("wk", [128, DT * 256], BF16, kind="ExternalInput").ap()
    wqv_d = nc.dram_tensor("wqv", [128, DT * 512], BF16,
                           kind="ExternalInput").ap()
    wo_d = nc.dram_tensor("wo", [256, DIM], BF16, kind="ExternalInput").ap()
    out_d = nc.dram_tensor("out", [N, DIM], BF16, kind="ExternalOutput").ap()

    with tile.TileContext(nc) as tc:
        _emit_kernel(tc, xtp_d, wk_d, wqv_d, wo_d, out_d)
    nc.compile()
    _CACHED_NC = nc
    return nc


def _in_maps(x, w_qkv, w_out):
    import ml_dtypes

    bf = ml_dtypes.bfloat16

    def pack(block):
        # [1024, W] (contraction-major) -> [128, DT*W]: row p holds the
        # per-dt blocks back to back, matching the SBUF [128, DT, W] tiles.
        w = block.shape[1]
        return block.reshape(DT, 128, w).transpose(1, 0, 2).reshape(128, DT * w)

    maps = []
    for c in range(N_CORES):
        b, g = divmod(c, 4)
        cols = slice(256 * g, 256 * (g + 1))
        xt = x[b].T  # [DIM, N]
        xtp = np.concatenate(
            [pack(xt[:, 512 * ch:512 * (ch + 1)]) for ch in range(CH)], axis=1
        )
        wqv = np.concatenate(
            [w_qkv[:, cols], w_qkv[:, 2 * INNER:][:, cols]], axis=1
        )
        maps.append(
            {
                "xtp": np.ascontiguousarray(xtp.astype(bf)),
                "wk": np.ascontiguousarray(
                    pack(w_qkv[:, INNER:][:, cols]).astype(bf)
                ),
                "wqv": np.ascontiguousarray(pack(wqv).astype(bf)),
                "wo": np.ascontiguousarray(w_out[cols, :].astype(bf)),
            }
        )
    return maps


def _run(x, w_qkv, w_out, b_out, trace=False):
    nc = _build()
    res = run_bass_kernel_spmd(
        nc, _in_maps(x, w_qkv, w_out), list(range(N_CORES)), trace=trace
    )
    partials = np.stack(
        [np.asarray(res.results[c]["out"], dtype=np.float32)
         for c in range(N_CORES)]
    )
    out = np.empty((B, N, DIM), dtype=np.float32)
    for b in range(B):
        out[b] = partials[4 * b:4 * b + 4].sum(axis=0) + b_out
    return out, res


def kernel(x, w_qkv, w_out, b_out):
    out, _ = _run(
        np.asarray(x, dtype=np.float32),
        np.asarray(w_qkv, dtype=np.float32),
        np.asarray(w_out, dtype=np.float32),
        np.asarray(b_out, dtype=np.float32),
    )
    return out
